# revision 1
# baseline (speedup 1.0000x reference)
"""Trainium2 Bass kernel for a 1-layer LSTM (B=2048, T=512, I=4, H=64) + FC (O=4).

Sharding: data-parallel over batch across 8 NeuronCores (256 examples/core);
the tiny LSTM/FC weights are replicated.

On-core layout ("transposed state"): SBUF partitions carry gate/hidden rows,
the free dimension carries batch.  The 256 local examples form two groups of
128; the two groups are stacked in the partition dimension (group 0 -> rows
0-63, group 1 -> rows 64-127) so ScalarE/VectorE instructions run with all
128 lanes busy and one instruction advances both groups.

Recurrent step t (lockstep over both groups, batch N=128 per group):
  z_g = [h_g (rows 0-63); ones (row 64); x_t^T (rows 65-68)]   # SBUF [69,128]
  8 matmuls (4 gate chunks x 2 groups), K=69, M=64, N=128:
      psA[128, 384] = [i | f | o]   (both groups stacked in partitions)
      psB[128, 128] = g-chunk
  sact = sigmoid(psA); tg = tanh(psB)          # 2 ScalarE instrs
  u = si*tg ; w = sf*c ; c = u + w             # 3 VectorE instrs [128,128]
  tc = tanh(c)                                 # 1 ScalarE instr
  h_g = so_g * tc_g  -> rows 0-63 of the other z buffer (group 1 needs a
      cross-quadrant partition shift, done as two 32-partition VectorE ops)

The input x is pre-transposed on the host to xT[T, I, B_local] so the
per-step x DMA is 4 contiguous rows.  Bias enters through the ones row of z;
the FC bias through the same ones row at the end.
"""

from contextlib import ExitStack

import numpy as np

import concourse.bass as bass
import concourse.tile as tile
from concourse import bacc, mybir
from concourse.bass_utils import run_bass_kernel_spmd

F32 = mybir.dt.float32
BF16 = mybir.dt.bfloat16
AF = mybir.ActivationFunctionType

H, I, O = 64, 4, 4
B, T_FULL = 2048, 512
NCORES = 8
BLOC = B // NCORES          # 256 examples per core
NG = 128                    # batch per group (2 groups per core)
KZ = H + 1 + I              # 69 rows of z: h, ones, x

# bf16 compute (matmuls, activations, cell state) keeps max rel err ~4e-3
# (measured against an f64 oracle) while roughly halving VectorE time.
USE_BF16 = True


def build_nc(T=T_FULL, use_bf16=None):
    if use_bf16 is None:
        use_bf16 = USE_BF16
    DT = BF16 if use_bf16 else F32
    nc = bacc.Bacc(
        "TRN2",
        target_bir_lowering=False,
        debug=False,
        enable_asserts=False,
        num_devices=NCORES,
    )

    xT = nc.dram_tensor("xT", [T, I, BLOC], DT, kind="ExternalInput")
    wz = nc.dram_tensor("wz", [KZ, 4, H], DT, kind="ExternalInput")
    wz2 = nc.dram_tensor("wz2", [2 * H, 4, H], DT, kind="ExternalInput")
    wfc = nc.dram_tensor("wfc", [KZ, O], DT, kind="ExternalInput")
    wfc2 = nc.dram_tensor("wfc2", [2 * H, O], DT, kind="ExternalInput")
    out = nc.dram_tensor("out", [2, O, NG], F32, kind="ExternalOutput")

    with tile.TileContext(nc) as tc, ExitStack() as ctx:
        persist = ctx.enter_context(tc.tile_pool(name="persist", bufs=1))
        acts = ctx.enter_context(tc.tile_pool(name="acts", bufs=3))
        temps = ctx.enter_context(tc.tile_pool(name="temps", bufs=3))
        psum = ctx.enter_context(tc.tile_pool(name="psum", bufs=2, space="PSUM"))

        wz_sb = persist.tile([KZ, 4, H], DT, tag="wz")
        nc.sync.dma_start(wz_sb[:], wz[:])
        wz2_sb = persist.tile([2 * H, 4, H], DT, tag="wz2")
        nc.sync.dma_start(wz2_sb[:], wz2[:])
        wfc_sb = persist.tile([KZ, O], DT, tag="wfc")
        nc.sync.dma_start(wfc_sb[:], wfc[:])
        wfc2_sb = persist.tile([2 * H, O], DT, tag="wfc2")
        nc.sync.dma_start(wfc2_sb[:], wfc2[:])

        # Persistent state: cell state (both groups stacked) and the two
        # double-buffered z tiles per group.  Group 0's z is [h; 1; x] (K=69,
        # h in partitions 0-63); group 1's is [1; x; zeros; h] (K=128, h in
        # partitions 64-127, zero rows cost nothing on the PE) so BOTH h
        # updates write the same partitions their operands live in.
        c_st = persist.tile([2 * H, NG], DT, tag="c")
        nc.vector.memset(c_st[:], 0.0)
        zbuf = []
        for j in range(2):
            z = persist.tile([KZ, NG], DT, tag=f"z0{j}")
            nc.vector.memset(z[0:H, :], 0.0)        # h0 = 0
            nc.vector.memset(z[H : H + 1, :], 1.0)  # ones row
            zbuf.append(z)
        zbuf2 = []
        for j in range(2):
            z = persist.tile([2 * H, NG], DT, tag=f"z1{j}")
            nc.vector.memset(z[:], 0.0)             # zeros rows + h0 = 0
            nc.vector.memset(z[0:1, :], 1.0)        # ones row (row 0)
            zbuf2.append(z)

        for t in range(T):
            zc = [zbuf[t % 2], zbuf2[t % 2]]
            zn = [zbuf[(t + 1) % 2], zbuf2[(t + 1) % 2]]

            # x_t for this step
            nc.sync.dma_start(zc[0][H + 1 : KZ, :], xT[t, :, 0:NG])
            nc.sync.dma_start(zc[1][1 : 1 + I, :], xT[t, :, NG : 2 * NG])

            # g-chunk first so tanh(g) is off ScalarE before sigmoid needs it
            psB = psum.tile([2 * H, NG], F32, tag="psB")      # g-chunk
            psA = psum.tile([2 * H, 3 * NG], F32, tag="psA")  # [i | f | o]
            wzs = [wz_sb, wz2_sb]
            for g in range(2):
                gp = slice(g * H, (g + 1) * H)
                nc.tensor.matmul(
                    psB[gp, :], wzs[g][:, 2, :], zc[g][:], start=True, stop=True
                )
            for g in range(2):
                gp = slice(g * H, (g + 1) * H)
                for ci, ch in enumerate((0, 1, 3)):  # i, f, o chunks
                    nc.tensor.matmul(
                        psA[gp, ci * NG : (ci + 1) * NG],
                        wzs[g][:, ch, :],
                        zc[g][:],
                        start=True,
                        stop=True,
                    )

            tg = acts.tile([2 * H, NG], DT, tag="tg")
            nc.scalar.activation(tg[:], psB[:], AF.Tanh)
            sact = acts.tile([2 * H, 3 * NG], DT, tag="sact")
            nc.scalar.activation(sact[:], psA[:], AF.Sigmoid)

            si = sact[:, 0:NG]
            sf = sact[:, NG : 2 * NG]
            so = sact[:, 2 * NG : 3 * NG]

            u = temps.tile([2 * H, NG], DT, tag="u")
            nc.vector.tensor_mul(u[:], si, tg[:])
            w = temps.tile([2 * H, NG], DT, tag="w")
            nc.vector.tensor_mul(w[:], sf, c_st[:])
            nc.vector.tensor_add(c_st[:], u[:], w[:])

            tcs = acts.tile([2 * H, NG], DT, tag="tc")
            nc.scalar.activation(tcs[:], c_st[:], AF.Tanh)

            # h updates: each group writes the partitions it already lives in.
            nc.vector.tensor_mul(zn[0][0:H, :], so[0:H, :], tcs[0:H, :])
            nc.vector.tensor_mul(zn[1][H : 2 * H, :], so[H:, :], tcs[H:, :])

        # Final FC (bias added on the host during the gather).
        zf0, zf1 = zbuf[T % 2], zbuf2[T % 2]
        fc_ps0 = psum.tile([O, NG], F32, tag="fc0")
        nc.tensor.matmul(fc_ps0[:], wfc_sb[:], zf0[:], start=True, stop=True)
        fc_ps1 = psum.tile([O, NG], F32, tag="fc1")
        nc.tensor.matmul(fc_ps1[:], wfc2_sb[:], zf1[:], start=True, stop=True)
        for g, fc_ps in enumerate((fc_ps0, fc_ps1)):
            fc_sb = temps.tile([O, NG], F32, tag="fcsb")
            nc.vector.tensor_copy(fc_sb[:], fc_ps[:])
            nc.sync.dma_start(out[g], fc_sb[:])

    nc.compile()
    return nc


def prep_weights(W_ih, W_hh, b_ih, b_hh, W_fc, b_fc):
    bsum = (b_ih + b_hh).astype(np.float32)
    # group 0 z rows: [h (64); ones (1); x (4)] -> [W_hh^T; b; W_ih^T]
    wz = np.empty((KZ, 4, H), np.float32)
    # group 1 z rows: [ones (1); x (4); zeros (59); h (64)]
    wz2 = np.zeros((2 * H, 4, H), np.float32)
    for ch in range(4):
        r = slice(ch * H, (ch + 1) * H)
        wz[0:H, ch, :] = W_hh[r].T
        wz[H, ch, :] = bsum[r]
        wz[H + 1 :, ch, :] = W_ih[r].T
        wz2[0, ch, :] = bsum[r]
        wz2[1 : 1 + I, ch, :] = W_ih[r].T
        wz2[H:, ch, :] = W_hh[r].T
    wfc = np.zeros((KZ, O), np.float32)
    wfc[0:H] = W_fc.T
    wfc2 = np.zeros((2 * H, O), np.float32)
    wfc2[H:] = W_fc.T
    return wz, wz2, wfc, wfc2


def make_in_maps(x, W_ih, W_hh, b_ih, b_hh, W_fc, b_fc, T=T_FULL, use_bf16=None):
    import ml_dtypes

    if use_bf16 is None:
        use_bf16 = USE_BF16
    npdt = ml_dtypes.bfloat16 if use_bf16 else np.float32
    wz, wz2, wfc, wfc2 = prep_weights(W_ih, W_hh, b_ih, b_hh, W_fc, b_fc)
    wz, wz2, wfc, wfc2 = (a.astype(npdt) for a in (wz, wz2, wfc, wfc2))
    in_maps = []
    for core in range(NCORES):
        xc = x[core * BLOC : (core + 1) * BLOC, :T, :]  # [BLOC, T, I]
        xTc = np.ascontiguousarray(xc.transpose(1, 2, 0)).astype(npdt)
        in_maps.append({"xT": xTc, "wz": wz, "wz2": wz2, "wfc": wfc, "wfc2": wfc2})
    return in_maps


_CACHED_NC = None


def kernel(x, W_ih, W_hh, b_ih, b_hh, W_fc, b_fc):
    global _CACHED_NC
    x = np.asarray(x, np.float32)
    args = [np.asarray(a, np.float32) for a in (W_ih, W_hh, b_ih, b_hh, W_fc, b_fc)]
    if _CACHED_NC is None:
        _CACHED_NC = build_nc()
    nc = _CACHED_NC
    in_maps = make_in_maps(x, *args)
    res = run_bass_kernel_spmd(nc, in_maps, core_ids=list(range(NCORES)))
    b_fc = args[5]
    full = np.empty((1, B, O), np.float32)
    for core in range(NCORES):
        oc = res.results[core]["out"]  # [2, O, NG]
        for g in range(2):
            lo = core * BLOC + g * NG
            full[0, lo : lo + NG, :] = oc[g].T + b_fc
    return full



# revision 2
# speedup vs baseline: 1.0097x; 1.0097x over previous
"""Trainium2 Bass kernel v4: 1-layer LSTM (B=2048, T=512, I=4, H=64) + FC (O=4).

Data-parallel over batch across 8 cores (256 examples/core).  The kernel is
LATENCY-bound: wall = T * per-step-chain.  All 256 local examples run in ONE
chain (2 slabs of 128 examples stacked in the partition dim, block-diagonal
[128,128] stationaries so one matmul serves both slabs).

Per-step chain (F=128):
  MM1(g) -> tanh_g -> (sigma_fi ||) [w|u] -> c' -> tanh_c -> h' -> MM1(t+1)

Key latency tricks:
  - x-side gates ([b;Wih] @ [1;x], K=9) PREFETCHED into the next step's psum
    tiles during the current step (PE is idle then); only the 4 Whh@h
    matmuls sit on the chain, g' first.
  - SEPARATE full-bank psum tiles per activation reader (psG / psFI / psO) so
    tanh_g's semaphore fires right after MM1(g), not after the whole phase.
  - tanh_g writes directly into the [c | tg] parent tile so ONE 2F-wide DVE
    multiply computes [w|u] = [sf|si] * [c|tg]; c' = w + u is one add.
  - sigma_o runs on Act between sigma_fi and tanh_c (needed only by h').
  - x for ALL steps prebuilt in SBUF (one DMA at start; zero per-step DMA).
"""

from contextlib import ExitStack

import numpy as np

import concourse.bass as bass
import concourse.tile as tile
from concourse import bacc, mybir
from concourse.bass_utils import run_bass_kernel_spmd

F32 = mybir.dt.float32
BF16 = mybir.dt.bfloat16
AF = mybir.ActivationFunctionType
ALU = mybir.AluOpType

H, I, O = 64, 4, 4
B, T_FULL = 2048, 512
NCORES = 8
BLOC = B // NCORES      # 256 per core
F = 128                 # examples per slab; 2 slabs stacked in partitions
KX = 1 + 2 * I          # xz rows: [1; x_slab0; x_slab1]

# weight-array order: [g, f, i, o] (chain emission order)
WSLOTS = ("g", "f", "i", "o")
GATE_IDX = {"i": 0, "f": 1, "g": 2, "o": 3}   # PyTorch gate order


def build_nc(T=T_FULL):
    nc = bacc.Bacc(
        "TRN2",
        target_bir_lowering=False,
        debug=False,
        enable_asserts=False,
        num_devices=NCORES,
    )

    xz = nc.dram_tensor("xz", [KX, T, F], BF16, kind="ExternalInput")
    whh = nc.dram_tensor("whh", [2 * H, 4, 2 * H], BF16, kind="ExternalInput")
    wih = nc.dram_tensor("wih", [KX, 4, 2 * H], BF16, kind="ExternalInput")
    wfc = nc.dram_tensor("wfc", [2 * H, 2 * O], BF16, kind="ExternalInput")
    out = nc.dram_tensor("out", [2 * O, F], F32, kind="ExternalOutput")

    with tile.TileContext(nc) as tc, ExitStack() as ctx:
        persist = ctx.enter_context(tc.tile_pool(name="persist", bufs=1))
        psum = ctx.enter_context(tc.tile_pool(name="psum", bufs=1, space="PSUM"))

        xz_sb = persist.tile([KX, T, F], BF16, tag="xz")
        nc.sync.dma_start(xz_sb[:], xz[:])
        whh_sb = [persist.tile([2 * H, 2 * H], BF16, tag=f"whh{s}", name=f"whh{s}")
                  for s in WSLOTS]
        wih_sb = [persist.tile([KX, 2 * H], BF16, tag=f"wih{s}", name=f"wih{s}")
                  for s in WSLOTS]
        for k in range(4):
            nc.sync.dma_start(whh_sb[k][:], whh[:, k, :])
            nc.sync.dma_start(wih_sb[k][:], wih[:, k, :])
        wfc_sb = persist.tile([2 * H, 2 * O], BF16, tag="wfc")
        nc.sync.dma_start(wfc_sb[:], wfc[:])

        # full-bank psum tiles (512 f32 cols) so tiles never share a bank:
        # per parity: G (tanh_g), FI (sigma_fi, uses 2F cols), O (sigma_o)
        psG = [psum.tile([2 * H, 512], F32, tag=f"psG{j}", name=f"psG{j}")
               for j in range(2)]
        psFI = [psum.tile([2 * H, 512], F32, tag=f"psFI{j}", name=f"psFI{j}")
                for j in range(2)]
        psO = [psum.tile([2 * H, 512], F32, tag=f"psO{j}", name=f"psO{j}")
               for j in range(2)]

        def tiles2(shape, dt, nm):
            return [persist.tile(shape, dt, tag=f"{nm}{j}", name=f"{nm}{j}")
                    for j in range(2)]

        h = tiles2([2 * H, F], BF16, "h")
        ct = tiles2([2 * H, 2 * F], BF16, "ct")    # [c | tg]
        cst = [ct[j][:, 0:F] for j in range(2)]
        tgt = [ct[j][:, F : 2 * F] for j in range(2)]
        sfi = tiles2([2 * H, 2 * F], BF16, "sfi")  # [sf | si]
        sO = tiles2([2 * H, F], BF16, "sO")
        wu = tiles2([2 * H, 2 * F], BF16, "wu")    # [w | u]
        tct = tiles2([2 * H, F], BF16, "tc")

        nc.vector.memset(h[0][:], 0.0)
        nc.vector.memset(cst[0], 0.0)

        # (tile, col-slice, weight-idx) per slot name
        def slot(nm, j):
            if nm == "g":
                return psG[j][:, 0:F], 0
            if nm == "f":
                return psFI[j][:, 0:F], 1
            if nm == "i":
                return psFI[j][:, F : 2 * F], 2
            return psO[j][:, 0:F], 3

        def prefetch(j, t):
            # x-side gates for step t into parity-j psum tiles.
            # start=True only on the first MM2 touching each bank.
            for nm, st in (("g", True), ("f", True), ("i", False), ("o", True)):
                ap, k = slot(nm, j)
                nc.tensor.matmul(ap, wih_sb[k][:], xz_sb[:, t, :],
                                 start=st, stop=False, skip_group_check=True)

        prefetch(0, 0)

        for t in range(T):
            cur, nxt = t % 2, (t + 1) % 2
            # chain MMs (g first), accumulate onto prefetched x-gates
            for nm in WSLOTS:
                ap, k = slot(nm, cur)
                nc.tensor.matmul(ap, whh_sb[k][:], h[cur][:],
                                 start=False, stop=True, skip_group_check=True)
            if t + 1 < T:
                prefetch(nxt, t + 1)
            # Act: tanh_g (-> tg half of ct), sigma_fi, sigma_o, tanh_c
            nc.scalar.activation(tgt[cur], psG[cur][:, 0:F], AF.Tanh)
            nc.scalar.activation(sfi[cur][:], psFI[cur][:, 0 : 2 * F], AF.Sigmoid)
            nc.scalar.activation(sO[cur][:], psO[cur][:, 0:F], AF.Sigmoid)
            # DVE: [w | u] = [sf | si] * [c | tg], then c' = w + u
            nc.vector.tensor_mul(wu[cur][:], sfi[cur][:], ct[cur][:])
            nc.vector.tensor_add(cst[nxt], wu[cur][:, 0:F], wu[cur][:, F : 2 * F])
            nc.scalar.activation(tct[cur][:], cst[nxt], AF.Tanh)
            nc.vector.tensor_mul(h[nxt][:], sO[cur][:], tct[cur][:])

        fc_ps = psum.tile([2 * O, F], F32, tag="fc")
        nc.tensor.matmul(fc_ps[:], wfc_sb[:], h[T % 2][:], start=True, stop=True)
        fc_sb = persist.tile([2 * O, F], F32, tag="fcsb")
        nc.vector.tensor_copy(fc_sb[:], fc_ps[:])
        nc.sync.dma_start(out[:], fc_sb[:])

    nc.compile()
    return nc


def prep_weights(W_ih, W_hh, b_ih, b_hh, W_fc):
    bsum = (b_ih + b_hh).astype(np.float32)
    whh = np.zeros((2 * H, 4, 2 * H), np.float32)
    wih = np.zeros((KX, 4, 2 * H), np.float32)
    for k, nm in enumerate(WSLOTS):
        g = GATE_IDX[nm]
        rows = slice(g * H, (g + 1) * H)
        wt = W_hh[rows].T
        whh[0:H, k, 0:H] = wt
        whh[H:, k, H:] = wt
        wih[0, k, :] = np.concatenate([bsum[rows]] * 2)
        wih[1 : 1 + I, k, 0:H] = W_ih[rows].T
        wih[1 + I :, k, H:] = W_ih[rows].T
    wfc = np.zeros((2 * H, 2 * O), np.float32)
    wfc[0:H, 0:O] = W_fc.T
    wfc[H:, O:] = W_fc.T
    return whh, wih, wfc


def make_in_maps(x, W_ih, W_hh, b_ih, b_hh, W_fc, b_fc, T=T_FULL):
    import ml_dtypes

    bf = ml_dtypes.bfloat16
    whh, wih, wfc = prep_weights(W_ih, W_hh, b_ih, b_hh, W_fc)
    whh, wih, wfc = (a.astype(bf) for a in (whh, wih, wfc))
    in_maps = []
    for core in range(NCORES):
        lo = core * BLOC
        xc = x[lo : lo + BLOC, :T, :]           # [256, T, I]
        xzv = np.empty((KX, T, F), np.float32)
        xzv[0] = 1.0
        xzv[1 : 1 + I] = xc[:F].transpose(2, 1, 0)
        xzv[1 + I :] = xc[F:].transpose(2, 1, 0)
        in_maps.append({"whh": whh, "wih": wih, "wfc": wfc,
                        "xz": np.ascontiguousarray(xzv).astype(bf)})
    return in_maps


_CACHED_NC = None


def kernel(x, W_ih, W_hh, b_ih, b_hh, W_fc, b_fc):
    global _CACHED_NC
    x = np.asarray(x, np.float32)
    args = [np.asarray(a, np.float32) for a in (W_ih, W_hh, b_ih, b_hh, W_fc, b_fc)]
    if _CACHED_NC is None:
        _CACHED_NC = build_nc()
    in_maps = make_in_maps(x, *args)
    res = run_bass_kernel_spmd(_CACHED_NC, in_maps, core_ids=list(range(NCORES)))
    return assemble(res, args[5])


def assemble(res, b_fc):
    full = np.empty((1, B, O), np.float32)
    for core in range(NCORES):
        oc = res.results[core]["out"]       # [2*O, F]
        for s in range(2):
            lo = core * BLOC + s * F
            full[0, lo : lo + F, :] = oc[s * O : (s + 1) * O, :].T + b_fc
    return full


# revision 3
# speedup vs baseline: 1.0149x; 1.0051x over previous
"""Trainium2 Bass kernel v4: 1-layer LSTM (B=2048, T=512, I=4, H=64) + FC (O=4).

Data-parallel over batch across 8 cores (256 examples/core).  The kernel is
LATENCY-bound: wall = T * per-step-chain.  All 256 local examples run in ONE
chain (2 slabs of 128 examples stacked in the partition dim, block-diagonal
[128,128] stationaries so one matmul serves both slabs).

Per-step chain (F=128):
  MM1(g) -> tanh_g -> (sigma_fi ||) [w|u] -> c' -> tanh_c -> h' -> MM1(t+1)

Key latency tricks:
  - x-side gates ([b;Wih] @ [1;x], K=9) PREFETCHED into the next step's psum
    tiles during the current step (PE is idle then); only the 4 Whh@h
    matmuls sit on the chain, g' first.
  - SEPARATE full-bank psum tiles per activation reader (psG / psFI / psO) so
    tanh_g's semaphore fires right after MM1(g), not after the whole phase.
  - tanh_g writes directly into the [c | tg] parent tile so ONE 2F-wide DVE
    multiply computes [w|u] = [sf|si] * [c|tg]; c' = w + u is one add.
  - sigma_o runs on Act between sigma_fi and tanh_c (needed only by h').
  - x for ALL steps prebuilt in SBUF (one DMA at start; zero per-step DMA).
"""

from contextlib import ExitStack

import numpy as np

import concourse.bass as bass
import concourse.tile as tile
from concourse import bacc, mybir
from concourse.bass_utils import run_bass_kernel_spmd

F32 = mybir.dt.float32
BF16 = mybir.dt.bfloat16
AF = mybir.ActivationFunctionType
ALU = mybir.AluOpType

H, I, O = 64, 4, 4
B, T_FULL = 2048, 512
NCORES = 8
BLOC = B // NCORES      # 256 per core
F = 128                 # examples per slab; 2 slabs stacked in partitions
KX = 1 + 2 * I          # xz rows: [1; x_slab0; x_slab1]

# weight-array order: [g, f, i, o] (chain emission order)
WSLOTS = ("g", "f", "i", "o")
GATE_IDX = {"i": 0, "f": 1, "g": 2, "o": 3}   # PyTorch gate order


def build_nc(T=T_FULL, n_fill=0, split_mm=False, split_g=False, pace_o=0):
    nc = bacc.Bacc(
        "TRN2",
        target_bir_lowering=False,
        debug=False,
        enable_asserts=False,
        num_devices=NCORES,
    )

    xz = nc.dram_tensor("xz", [KX, T, F], BF16, kind="ExternalInput")
    whh = nc.dram_tensor("whh", [2 * H, 4, 2 * H], BF16, kind="ExternalInput")
    wih = nc.dram_tensor("wih", [KX, 4, 2 * H], BF16, kind="ExternalInput")
    wfc = nc.dram_tensor("wfc", [2 * H, 2 * O], BF16, kind="ExternalInput")
    out = nc.dram_tensor("out", [2 * O, F], F32, kind="ExternalOutput")

    with tile.TileContext(nc) as tc, ExitStack() as ctx:
        persist = ctx.enter_context(tc.tile_pool(name="persist", bufs=1))
        psum = ctx.enter_context(tc.tile_pool(name="psum", bufs=1, space="PSUM"))

        HEAD = min(16, T)
        whh_sb = [persist.tile([2 * H, 2 * H], BF16, tag=f"whh{s}", name=f"whh{s}")
                  for s in WSLOTS]
        wih_sb = [persist.tile([KX, 2 * H], BF16, tag=f"wih{s}", name=f"wih{s}")
                  for s in WSLOTS]
        for k in range(4):
            nc.sync.dma_start(whh_sb[k][:], whh[:, k, :])
            nc.sync.dma_start(wih_sb[k][:], wih[:, k, :])
        wfc_sb = persist.tile([2 * H, 2 * O], BF16, tag="wfc")
        nc.sync.dma_start(wfc_sb[:], wfc[:])
        # small head tile unblocks step 0 while the full xz streams in
        xz_head = persist.tile([KX, HEAD, F], BF16, tag="xzh", name="xzh")
        nc.sync.dma_start(xz_head[:], xz[:, 0:HEAD, :])
        xz_sb = persist.tile([KX, T, F], BF16, tag="xz")
        nc.sync.dma_start(xz_sb[:], xz[:])

        # full-bank psum tiles (512 f32 cols) so tiles never share a bank:
        # per parity: G (tanh_g), FI (sigma_fi, uses 2F cols), O (sigma_o)
        psG = [psum.tile([2 * H, 512], F32, tag=f"psG{j}", name=f"psG{j}")
               for j in range(2)]
        psFI = [psum.tile([2 * H, 512], F32, tag=f"psFI{j}", name=f"psFI{j}")
                for j in range(2)]
        psO = [psum.tile([2 * H, 512], F32, tag=f"psO{j}", name=f"psO{j}")
               for j in range(2)]
        if n_fill:
            fill_ps = psum.tile([2 * H, 512], F32, tag="fill", name="fill_ps")
            fill_rhs = persist.tile([2 * H, 256], BF16, tag="fillr", name="fill_rhs")
            nc.vector.memset(fill_rhs[:], 0.0)

        def tiles2(shape, dt, nm):
            return [persist.tile(shape, dt, tag=f"{nm}{j}", name=f"{nm}{j}")
                    for j in range(2)]

        h = tiles2([2 * H, F], BF16, "h")
        ct = tiles2([2 * H, 2 * F], BF16, "ct")    # [c | tg]
        cst = [ct[j][:, 0:F] for j in range(2)]
        tgt = [ct[j][:, F : 2 * F] for j in range(2)]
        sfi = tiles2([2 * H, 2 * F], BF16, "sfi")  # [sf | si]
        sO = tiles2([2 * H, 512 if pace_o else F], BF16, "sO")
        wu = tiles2([2 * H, 2 * F], BF16, "wu")    # [w | u]
        tct = tiles2([2 * H, F], BF16, "tc")

        nc.vector.memset(h[0][:], 0.0)
        nc.vector.memset(cst[0], 0.0)
        if pace_o:
            for j in range(2):
                nc.vector.memset(psO[j][:, F:pace_o], 0.0)

        # (tile, col-slice, weight-idx) per slot name
        def slot(nm, j):
            if nm == "g":
                return psG[j][:, 0:F], 0
            if nm == "f":
                return psFI[j][:, 0:F], 1
            if nm == "i":
                return psFI[j][:, F : 2 * F], 2
            return psO[j][:, 0:F], 3

        def prefetch(j, t):
            # x-side gates for step t into parity-j psum tiles.
            # start=True only on the first MM2 touching each bank.
            xs = xz_head[:, t, :] if t < HEAD else xz_sb[:, t, :]
            for nm, st in (("g", True), ("f", True), ("i", False), ("o", True)):
                ap, k = slot(nm, j)
                nc.tensor.matmul(ap, wih_sb[k][:], xs,
                                 start=st, stop=False, skip_group_check=True)

        prefetch(0, 0)

        for t in range(T):
            cur, nxt = t % 2, (t + 1) % 2
            # chain MMs (g first), accumulate onto prefetched x-gates
            for nm in WSLOTS:
                ap, k = slot(nm, cur)
                if split_g and nm == "g":
                    nc.tensor.matmul(ap[0:H, :], whh_sb[k][0:H, 0:H],
                                     h[cur][0:H, :], start=False, stop=True,
                                     skip_group_check=True)
                    nc.tensor.matmul(ap[H:, :], whh_sb[k][H:, H:],
                                     h[cur][H:, :], start=False, stop=True,
                                     skip_group_check=True)
                elif split_mm:
                    # block-diagonal: two independent K=64 halves, concurrent
                    # via tile_position (disjoint row/col groups)
                    nc.tensor.matmul(ap[0:H, :], whh_sb[k][0:H, 0:H],
                                     h[cur][0:H, :], start=False, stop=True,
                                     skip_group_check=True)
                    nc.tensor.matmul(ap[H:, :], whh_sb[k][H:, H:],
                                     h[cur][H:, :], start=False, stop=True,
                                     skip_group_check=True)
                else:
                    nc.tensor.matmul(ap, whh_sb[k][:], h[cur][:],
                                     start=False, stop=True, skip_group_check=True)
            if t + 1 < T:
                prefetch(nxt, t + 1)
            for fi in range(n_fill):
                nc.tensor.matmul(fill_ps[:, 0:256], whh_sb[fi % 4][:, 0:128],
                                 fill_rhs[:], start=True, stop=True,
                                 skip_group_check=True)
            # Act: tanh_g (-> tg half of ct), sigma_fi, sigma_o, tanh_c
            nc.scalar.activation(tgt[cur], psG[cur][:, 0:F], AF.Tanh)
            nc.scalar.activation(sfi[cur][:], psFI[cur][:, 0 : 2 * F], AF.Sigmoid)
            nc.scalar.activation(sO[cur][:, 0 : (pace_o or F)],
                                 psO[cur][:, 0 : (pace_o or F)], AF.Sigmoid)
            # DVE: [w | u] = [sf | si] * [c | tg], then c' = w + u
            nc.vector.tensor_mul(wu[cur][:], sfi[cur][:], ct[cur][:])
            nc.vector.tensor_add(cst[nxt], wu[cur][:, 0:F], wu[cur][:, F : 2 * F])
            nc.scalar.activation(tct[cur][:], cst[nxt], AF.Tanh)
            nc.vector.tensor_mul(h[nxt][:], sO[cur][:, 0:F], tct[cur][:])

        fc_ps = psum.tile([2 * O, F], F32, tag="fc")
        nc.tensor.matmul(fc_ps[:], wfc_sb[:], h[T % 2][:], start=True, stop=True)
        fc_sb = persist.tile([2 * O, F], F32, tag="fcsb")
        nc.vector.tensor_copy(fc_sb[:], fc_ps[:])
        nc.sync.dma_start(out[:], fc_sb[:])

    nc.compile()
    return nc


def prep_weights(W_ih, W_hh, b_ih, b_hh, W_fc):
    bsum = (b_ih + b_hh).astype(np.float32)
    whh = np.zeros((2 * H, 4, 2 * H), np.float32)
    wih = np.zeros((KX, 4, 2 * H), np.float32)
    for k, nm in enumerate(WSLOTS):
        g = GATE_IDX[nm]
        rows = slice(g * H, (g + 1) * H)
        wt = W_hh[rows].T
        whh[0:H, k, 0:H] = wt
        whh[H:, k, H:] = wt
        wih[0, k, :] = np.concatenate([bsum[rows]] * 2)
        wih[1 : 1 + I, k, 0:H] = W_ih[rows].T
        wih[1 + I :, k, H:] = W_ih[rows].T
    wfc = np.zeros((2 * H, 2 * O), np.float32)
    wfc[0:H, 0:O] = W_fc.T
    wfc[H:, O:] = W_fc.T
    return whh, wih, wfc


def make_in_maps(x, W_ih, W_hh, b_ih, b_hh, W_fc, b_fc, T=T_FULL):
    import ml_dtypes

    bf = ml_dtypes.bfloat16
    whh, wih, wfc = prep_weights(W_ih, W_hh, b_ih, b_hh, W_fc)
    whh, wih, wfc = (a.astype(bf) for a in (whh, wih, wfc))
    in_maps = []
    for core in range(NCORES):
        lo = core * BLOC
        xc = x[lo : lo + BLOC, :T, :]           # [256, T, I]
        xzv = np.empty((KX, T, F), np.float32)
        xzv[0] = 1.0
        xzv[1 : 1 + I] = xc[:F].transpose(2, 1, 0)
        xzv[1 + I :] = xc[F:].transpose(2, 1, 0)
        in_maps.append({"whh": whh, "wih": wih, "wfc": wfc,
                        "xz": np.ascontiguousarray(xzv).astype(bf)})
    return in_maps


_CACHED_NC = None


def kernel(x, W_ih, W_hh, b_ih, b_hh, W_fc, b_fc):
    global _CACHED_NC
    x = np.asarray(x, np.float32)
    args = [np.asarray(a, np.float32) for a in (W_ih, W_hh, b_ih, b_hh, W_fc, b_fc)]
    if _CACHED_NC is None:
        _CACHED_NC = build_nc()
    in_maps = make_in_maps(x, *args)
    res = run_bass_kernel_spmd(_CACHED_NC, in_maps, core_ids=list(range(NCORES)))
    return assemble(res, args[5])


def assemble(res, b_fc):
    full = np.empty((1, B, O), np.float32)
    for core in range(NCORES):
        oc = res.results[core]["out"]       # [2*O, F]
        for s in range(2):
            lo = core * BLOC + s * F
            full[0, lo : lo + F, :] = oc[s * O : (s + 1) * O, :].T + b_fc
    return full
